# revision 35
# baseline (speedup 1.0000x reference)
"""Multi-head attention (B=2, S=2048, D=1024, H=16) on 8 TRN2 NeuronCores.

Sharding: batch x head-group parallel. Core c handles batch b = c//4 and
heads 4*(c%4) .. 4*(c%4)+3.  Q/K/V projections are column-split per core,
Wo is row-split; the 4 partial [S, D] outputs per batch are summed on the
host (f16 partials, f32 accumulation + bo).

v5 design (vs. 304us baseline):
  - masks stored u8 in DRAM (16.8 MB/core) and cast u8->f16 by the SWDGE
    casting DMA; mask tiles are one TT-pair each ([128, 2, 1024], 4KB)
    with a 10-deep pool so the mask stream runs ~5 chunk-pairs ahead.
  - score matmuls for the two heads of a pair are row-tiled (K=64 rows
    0-63 / 64-127) and run concurrently in the PE array.
  - ScalarE's exp stream (~147us) is the critical engine.  The serial
    chain exp -> mask-mul -> attn@V -> next scores is broken by lagging
    attn@V two k-chunks behind the mask multiply, so the PE never waits
    on the current chunk's DVE work.
  - attn@V: pts stationary [128,128], N=65; 4 q-chunks share one PSUM
    bank ([128,4,65], Z in col 64; only the first matmul sets start=True
    since start clears the whole bank).  Eviction = reciprocal +
    stride-0-broadcast multiply into outh.
  - Wo projection pieces (transpose + 2 matmuls + copy + DMA per 128-q
    chunk) run in block-boundary windows on the freed attn@V PSUM slots;
    output partials are f16.
"""

import os
import sys

for _p in ("/opt/trn_rl_repo", "/root/.axon_site/_ro/trn_rl_repo"):
    if os.path.isdir(_p) and _p not in sys.path:
        sys.path.append(_p)

import numpy as np

import concourse.bass as bass
import concourse.tile as tile
from concourse import bacc, mybir
from concourse.bass_utils import run_bass_kernel_spmd

B, S, D, H = 2, 2048, 1024, 16
DH = D // H            # 64
HPC = 4                # heads per core
PAIRS = 2              # head pairs per core
N_CORES = 8
P = 128
NB = 512               # matmul free-dim block (one PSUM bank of fp32)
KC = S // P            # 16 k chunks
QB = 1024              # q block (2 blocks per core)
DC = D // P            # 8 contraction chunks for projections
SCALE = 1.0 / 8.0      # 1/sqrt(DH)

F32 = mybir.dt.float32
F16 = mybir.dt.float16
F8 = mybir.dt.float8e4
U8 = mybir.dt.uint8
WS = 1.0               # no weight scaling in the all-f16 configuration

Exp = mybir.ActivationFunctionType.Exp


def _build_attention_kernel(tc):
    nc = tc.nc
    qt = nc.dram_tensor("qt", [D, S], F16, kind="ExternalInput").ap()
    kt = nc.dram_tensor("kt", [D, S], F16, kind="ExternalInput").ap()
    vt = nc.dram_tensor("vt", [D, S], F16, kind="ExternalInput").ap()
    # inverted transposed mask: 1 = keep, 0 = masked; [head, k, q], u8
    invm = nc.dram_tensor("invm", [HPC, S, S], U8, kind="ExternalInput").ap()
    # host pre-rearranged weights (see _make_in_maps)
    wq = nc.dram_tensor("wq", [P, DC * HPC * DH], F16, kind="ExternalInput").ap()
    wk = nc.dram_tensor("wk", [P, DC * HPC * DH], F16, kind="ExternalInput").ap()
    wv = nc.dram_tensor("wv", [P, DC * HPC * DH], F16, kind="ExternalInput").ap()
    wo = nc.dram_tensor("wo", [P, PAIRS * D], F16, kind="ExternalInput").ap()
    bq = nc.dram_tensor("bq", [P, PAIRS], F32, kind="ExternalInput").ap()
    bk = nc.dram_tensor("bk", [P, PAIRS], F32, kind="ExternalInput").ap()
    bv = nc.dram_tensor("bv", [HPC * DH], F16, kind="ExternalInput").ap()
    out = nc.dram_tensor("out", [S, D], F16, kind="ExternalOutput").ap()

    with (
        tc.tile_pool(name="const", bufs=1) as constp,
        tc.tile_pool(name="wts", bufs=1) as wtsp,
        tc.tile_pool(name="proj", bufs=1) as projp,
        tc.tile_pool(name="xt", bufs=16) as xtp,
        tc.tile_pool(name="pts", bufs=2) as ptsp,
        tc.tile_pool(name="mask", bufs=5) as maskp,
        tc.tile_pool(name="rz", bufs=4) as rzp,
        tc.tile_pool(name="ostage", bufs=2) as ostagep,
        tc.tile_pool(name="po", bufs=4, space="PSUM") as pop,
        tc.tile_pool(name="ps2", bufs=2, space="PSUM") as psp2,
    ):
        # ---- constants ----
        ones_f = constp.tile([1, P], F32)
        nc.vector.memset(ones_f[:], 1.0)
        ones_row = constp.tile([1, P], F16)      # K=1 lhsT for v bias add
        nc.vector.memset(ones_row[:], 1.0)
        # warm-up stream: dependency-free matmuls (K=1, ones stationary) so
        # HAM lifts the PE clock to 2.4GHz before the projection matmuls
        # arrive; gated only on a memset, unlike the old identity-based
        # stream that waited ~9us for make_identity.
        warm = pop.tile([P, DH], F32, tag="po", name="warm")
        for _ in range(96):
            nc.tensor.matmul(
                warm[:], ones_row[:], ones_row[:, 0:DH], start=True,
                stop=True, skip_group_check=True,
            )
        ident = constp.tile([P, P], F16)
        from concourse.masks import make_identity
        make_identity(nc, ident[:])
        # trigger the exp ACT table load early, while DMA-bound
        dummy = constp.tile([1, P], F16)
        nc.scalar.activation(dummy[:], ones_f[:], Exp)

        # ---- weights / biases (contiguous layouts, scalar HWDGE ring) ----
        def load_w(name, w_ap, dt=F16):
            t = wtsp.tile([P, DC, HPC * DH], dt, tag=name)
            nc.scalar.dma_start(t[:], w_ap.rearrange("r (j c) -> r j c", j=DC))
            return t

        wk_sb = load_w("wk", wk)    # k first: khT projection starts first
        wq_sb = load_w("wq", wq)
        bq_sb = wtsp.tile([P, PAIRS], F32, tag="bq")
        nc.scalar.dma_start(bq_sb[:], bq)
        bk_sb = wtsp.tile([P, PAIRS], F32, tag="bk")
        nc.scalar.dma_start(bk_sb[:], bk)
        bv_sb = wtsp.tile([1, HPC * DH], F16, tag="bv")
        nc.scalar.dma_start(bv_sb[:], bv.rearrange("(o c) -> o c", o=1))
        # wv/wo loads are deferred into the first block's j-loop to keep
        # startup DMA bandwidth for the k/q/mask critical path
        wv_sb = wtsp.tile([P, DC, HPC * DH], F16, tag="wv")
        wo_sb = wtsp.tile([P, PAIRS, D], F16, tag="wo")

        def load_wv():
            nc.scalar.dma_start(
                wv_sb[:], wv.rearrange("r (j c) -> r j c", j=DC)
            )

        def load_wo():
            nc.scalar.dma_start(
                wo_sb[:], wo.rearrange("r (p n) -> r p n", p=PAIRS)
            )

        # ---- persistent activations ----
        qhT = projp.tile([P, PAIRS, S], F16, tag="qhT")
        khT = projp.tile([P, PAIRS, S], F16, tag="khT")
        vh1 = projp.tile([P, HPC, KC, DH + 1], F16, tag="vh1")
        nc.vector.memset(vh1[:, :, :, DH : DH + 1], WS)
        outh = projp.tile([P, S // P, HPC * DH], F16, tag="outh")
        outhT = projp.tile([P, PAIRS, S], F16, tag="outhT")

        # ---- q/k projection: per-S/2-half DMA load, per-512q mm piece ----
        def emit_qk_dmas(src, sh, eng=None, dt=F16, split=False, eng2=None):
            eng = eng or nc.sync
            eng2 = eng2 or eng
            c0 = sh * (S // 2)
            xts = []
            for j in range(DC):
                x_t = xtp.tile([P, S // 2], dt, name=f"x_{j}", tag="xt")
                if split:
                    # first 512 columns queued for all j first: the sq=0
                    # projection pieces (and so the first scores/exp) start
                    # after half the DMA bytes.
                    eng.dma_start(
                        x_t[:, 0:NB], src[j * P : (j + 1) * P, c0 : c0 + NB]
                    )
                else:
                    eng.dma_start(
                        x_t[:], src[j * P : (j + 1) * P, c0 : c0 + S // 2]
                    )
                xts.append(x_t)
            if split:
                for j in range(DC):
                    eng2.dma_start(
                        xts[j][:, NB : S // 2],
                        src[j * P : (j + 1) * P, c0 + NB : c0 + S // 2],
                    )
            return xts

        def emit_qk_mms(xts, w_sb, b_sb, dst, sh, sq):
            c0 = sh * (S // 2) + sq * NB
            ps = [
                pop.tile([P, NB], F32, tag="po", name=f"ps_proj{p}")
                for p in range(PAIRS)
            ]
            for j in range(DC):
                for p in range(PAIRS):
                    nc.tensor.matmul(
                        ps[p][:],
                        w_sb[:, j, p * P : (p + 1) * P],
                        xts[j][:, sq * NB : (sq + 1) * NB],
                        start=(j == 0),
                        stop=(j == DC - 1),
                    )
            for p in range(PAIRS):
                nc.vector.tensor_scalar_add(
                    dst[:, p, c0 : c0 + NB], ps[p][:], b_sb[:, p : p + 1]
                )

        # ---- v projection mms for k-chunks of one half ----
        def emit_v_mms(vts, sh, sq, kks=range(4)):
            for kk in kks:
                kidx = sh * 8 + sq * 4 + kk
                ps = psp2.tile([P, HPC * DH], F32, tag="ps2", name="ps_v")
                for j in range(DC):
                    nc.tensor.matmul(
                        ps[:],
                        vts[j][:, (sq * 4 + kk) * P : (sq * 4 + kk + 1) * P],
                        wv_sb[:, j, :],
                        start=(j == 0),
                        stop=False,
                    )
                nc.tensor.matmul(
                    ps[:], ones_row[:], bv_sb[:], start=False, stop=True
                )
                nc.vector.tensor_copy(
                    vh1[:, :, kidx, 0:DH],
                    ps[:].rearrange("r (h c) -> r h c", h=HPC),
                )

        invm_r = [invm[lh].rearrange("(j p) q -> p j q", p=P) for lh in range(HPC)]

        # ---- attention block machinery ----
        class Block:
            def __init__(self, pp, qb, eager_groups=1):
                self.pp, self.qb = pp, qb
                self.m_tiles = {}
                self.m_next = 0
                # per-chunk pts tiles: subtile tracking on a strided view of
                # one big tile collapses to whole-tile deps, which made the
                # next block's first exp wait on this block's LAST attn@V
                # reads. 17 bufs (vs 16 live) shift the slot mapping each
                # block so a new tile's slot held a long-dead chunk.
                self.pts = [
                    [
                        ptsp.tile([P, QB], F16, name=f"pts{h2}_{c}",
                                  tag=f"pts{h2}", bufs=17)
                        for c in range(KC)
                    ]
                    for h2 in range(2)
                ]
                self.po = {}       # (h2, qcg) -> psum tile
                self.attnv_next = 0
                self.ensure_masks(eager_groups)

            def ensure_masks(self, upto):
                # group-major, head-minor: slot reuse order matches the TT
                # consumption order; lazy emission keeps startup bandwidth
                # for the k/q/weight critical path.
                while self.m_next < min(upto, 4):
                    g = self.m_next
                    for h2 in range(2):
                        lh = self.pp * 2 + h2
                        m_t = maskp.tile(
                            [P, 4, QB], F16, name=f"m_{h2}_{g}", tag="m"
                        )
                        nc.gpsimd.dma_start(
                            m_t[:],
                            invm_r[lh][
                                :,
                                g * 4 : (g + 1) * 4,
                                self.qb * QB : (self.qb + 1) * QB,
                            ],
                        )
                        self.m_tiles[(h2, g)] = m_t
                    self.m_next += 1

            def scores(self, j):
                """Scores + exp for k-chunk j (both heads, row-tiled)."""
                pss = [
                    psp2.tile([P, QB], F32, tag="ps2", name=f"ps_s{h2}")
                    for h2 in range(2)
                ]
                for nh in range(2):
                    for h2 in range(2):
                        d0 = h2 * DH
                        q0 = self.qb * QB + nh * NB
                        nc.tensor.matmul(
                            pss[h2][:, nh * NB : (nh + 1) * NB],
                            khT[d0 : d0 + DH, self.pp, j * P : (j + 1) * P],
                            qhT[d0 : d0 + DH, self.pp, q0 : q0 + NB],
                            start=True,
                            stop=True,
                        )
                for h2 in range(2):
                    nc.scalar.activation(self.pts[h2][j][:], pss[h2][:], Exp)

            def mask(self, j):
                """Mask-multiply chunk pair (j-1, j), N=2048 per DVE op."""
                g, r = (j - 1) // 4, ((j - 1) % 4) // 2
                self.ensure_masks(g + 2)
                for h2 in range(2):
                    for dj in (0, 1):
                        sl = self.pts[h2][j - 1 + dj][:]
                        nc.vector.tensor_mul(
                            sl, sl,
                            self.m_tiles[(h2, g)][:, 2 * r + dj, :],
                        )

            def attnv_chunk(self, c, qc8s=range(8)):
                """attn@V matmuls for (already masked) k-chunk c."""
                for h2 in range(2):
                    lh = self.pp * 2 + h2
                    for qc8 in qc8s:
                        qcg, g = qc8 // 4, qc8 % 4
                        po = self.po.get((h2, qcg))
                        if po is None:
                            po = pop.tile(
                                [P, 4, DH + 1], F32, tag="po",
                                name=f"po{h2}{qcg}",
                            )
                            self.po[(h2, qcg)] = po
                        # start=True clears the whole PSUM bank: only the
                        # first matmul into the tile sets it; later groups'
                        # first writes use per-element has_written overwrite.
                        nc.tensor.matmul(
                            po[:, g, :],
                            self.pts[h2][c][:, qc8 * P : (qc8 + 1) * P],
                            vh1[:, lh, c, :],
                            start=(c == 0 and g == 0),
                            stop=(c == KC - 1),
                            skip_group_check=True,
                        )

            def attnv_upto(self, limit):
                while self.attnv_next < limit:
                    self.attnv_chunk(self.attnv_next)
                    self.attnv_next += 1

            def evict(self, qcgs=(0, 1)):
                """Normalize po accumulators into outh."""
                for h2 in range(2):
                    lh = self.pp * 2 + h2
                    for qcg in qcgs:
                        po = self.po[(h2, qcg)]
                        rzt = rzp.tile([P, 4], F32, name="rzt", tag="rz")
                        nc.vector.reciprocal(rzt[:], po[:, :, DH])
                        qc0 = self.qb * 8 + qcg * 4
                        nc.vector.tensor_mul(
                            outh[:, qc0 : qc0 + 4, lh * DH : (lh + 1) * DH],
                            po[:, 0:4, 0:DH],
                            rzt[:].unsqueeze(2).broadcast_to([P, 4, DH]),
                        )

        # Wo projection piece for one 128-q chunk (after outh[qc] complete).
        # psum comes from the po pool: call only in boundary/tail windows.
        def emit_wo_piece(qc):
            for p in range(PAIRS):
                tp = pop.tile([P, P], F16, tag="po", name="tp")
                nc.tensor.transpose(
                    tp[:], outh[:, qc, p * P : (p + 1) * P], ident[:]
                )
                nc.vector.tensor_copy(
                    outhT[:, p, qc * P : (qc + 1) * P], tp[:]
                )
            o_t = ostagep.tile([P, D], F16)
            for nb in range(D // NB):
                pf = pop.tile([P, NB], F32, tag="po", name="pf")
                for p in range(PAIRS):
                    nc.tensor.matmul(
                        pf[:],
                        outhT[:, p, qc * P : (qc + 1) * P],
                        wo_sb[:, p, nb * NB : (nb + 1) * NB],
                        start=(p == 0),
                        stop=(p == PAIRS - 1),
                    )
                nc.vector.tensor_copy(o_t[:, nb * NB : (nb + 1) * NB], pf[:])
            nc.sync.dma_start(out[qc * P : (qc + 1) * P, :], o_t[:])

        # ---- emission schedule ----
        # kt streams on the sync ring while qt+weights stream on the
        # scalar ring; the mask DMAs (gpsimd ring) are emitted after so
        # the k/q critical path gets first claim on the SDMA engines.
        # qt first quarter on the scalar ring (behind the weights), second
        # quarter leading the sync ring; kt's first quarter follows it on
        # sync: the minimal prefix for scores(0) -- k chunks 0-3 + the full
        # first q half -- is ~4MB split evenly across both rings.
        qts = emit_qk_dmas(qt, 0, nc.scalar, split=True, eng2=nc.sync)
        kts = emit_qk_dmas(kt, 0, nc.sync, split=True)
        # eager_groups=1: only the first mask group-pair competes with the
        # k/q/weight streams for startup HBM bandwidth; later groups are
        # fetched lazily two groups ahead of use.
        b00 = Block(0, 0, eager_groups=1)
        emit_qk_mms(kts, wk_sb, bk_sb, khT, 0, 0)
        emit_qk_mms(qts, wq_sb, bq_sb, qhT, 0, 0)
        emit_qk_mms(qts, wq_sb, bq_sb, qhT, 0, 1)

        # block (0,0): scores stream immediately; the k/q second halves
        # and the v projection are inserted piecewise into the exp-paced
        # j-loop (pieces sized ~1-3us so they never head-of-line-block the
        # exp stream for long); attn@V catches up 4 chunks per odd j from
        # j=9.
        nxt = None
        for j in range(KC):
            b00.scores(j)
            if j % 2 == 1:
                b00.mask(j)
            if j == 1:
                emit_qk_mms(kts, wk_sb, bk_sb, khT, 0, 1)
            if j == 2:
                kts = emit_qk_dmas(kt, 1, nc.sync)
                load_wv()
            if j == 4:
                qts = emit_qk_dmas(qt, 1, nc.scalar)
                vts1 = emit_qk_dmas(vt, 0, nc.sync)
            if j == 5:
                emit_qk_mms(kts, wk_sb, bk_sb, khT, 1, 0)
            if j == 7:
                emit_qk_mms(kts, wk_sb, bk_sb, khT, 1, 1)
            if j == 8:
                emit_v_mms(vts1, 0, 0, range(0, 2))
                vts2 = emit_qk_dmas(vt, 1, nc.scalar)
            if j == 9:
                emit_qk_mms(qts, wq_sb, bq_sb, qhT, 1, 0)
                load_wo()
            if j == 10:
                emit_v_mms(vts1, 0, 0, range(2, 4))
            if j == 11:
                emit_v_mms(vts1, 0, 1, range(0, 2))
            if j == 12:
                emit_v_mms(vts1, 0, 1, range(2, 4))
            if j == 13:
                emit_v_mms(vts2, 1, 0, range(0, 2))
                nxt = Block(1, 0, eager_groups=3)
            if j == 14:
                emit_v_mms(vts2, 1, 0, range(2, 4))
                emit_qk_mms(qts, wq_sb, bq_sb, qhT, 1, 1)
            if j == 15:
                emit_v_mms(vts2, 1, 1, range(0, 2))
            if j >= 11 and j % 2 == 1:
                b00.attnv_upto(min(4 * ((j - 9) // 2), j - 1))
        emit_v_mms(vts2, 1, 1, range(2, 4))
        b00.attnv_upto(KC)
        b00.evict()

        def emit_body(blk, next_spec, boundary_wo=(), finish=True,
                      early_transpose=()):
            nxt = None
            for j in range(KC):
                blk.scores(j)
                if j % 2 == 1:
                    blk.mask(j)
                    if j >= 3:
                        blk.attnv_upto(j - 2)
                if j in (1, 3) and boundary_wo:
                    # boundary window: previous q-block's Wo pieces run on
                    # the PE while this block's first exps drain.
                    half = len(boundary_wo) // 2
                    for qc in (boundary_wo[:half] if j == 1 else boundary_wo[half:]):
                        emit_wo_piece(qc)
                if j in (5, 7) and early_transpose:
                    # pair-0 columns of outh for this pair-1 block's q-range
                    # are already final: transpose them now, off the tail.
                    half = len(early_transpose) // 2
                    qcs = early_transpose[:half] if j == 5 else early_transpose[half:]
                    emit_wo_transposes(qcs, [0], use_ps2=True)
                if j == 13 and next_spec:
                    nxt = Block(*next_spec, eager_groups=3)
            if finish:
                blk.attnv_upto(KC)
                blk.evict()
            return nxt

        # transposes for outh -> outhT, one (qc, pair); pool selectable so
        # pair-0 transposes can run a block early (outh cols 0-127 are
        # written by the pair-0 blocks alone).
        def emit_wo_transposes(qcs, pairs, use_ps2=False):
            for qc in qcs:
                for p in pairs:
                    pool = psp2 if use_ps2 else pop
                    tp = pool.tile([P, P], F16,
                                   tag="ps2" if use_ps2 else "po", name="tp")
                    nc.tensor.transpose(
                        tp[:], outh[:, qc, p * P : (p + 1) * P], ident[:]
                    )
                    nc.vector.tensor_copy(
                        outhT[:, p, qc * P : (qc + 1) * P], tp[:]
                    )

        def emit_wo_mms(qcs, scalar_copy=False):
            Cp = mybir.ActivationFunctionType.Copy
            for qc in qcs:
                o_t = ostagep.tile([P, D], F16)
                for nb in range(D // NB):
                    pool = pop if nb == 0 else psp2
                    pf = pool.tile([P, NB], F32,
                                   tag="po" if pool is pop else "ps2",
                                   name="pf")
                    for p in range(PAIRS):
                        nc.tensor.matmul(
                            pf[:],
                            outhT[:, p, qc * P : (qc + 1) * P],
                            wo_sb[:, p, nb * NB : (nb + 1) * NB],
                            start=(p == 0),
                            stop=(p == PAIRS - 1),
                        )
                    dst = o_t[:, nb * NB : (nb + 1) * NB]
                    if scalar_copy and nb == 1:
                        nc.scalar.activation(dst, pf[:], Cp)
                    else:
                        nc.vector.tensor_copy(dst, pf[:])
                nc.sync.dma_start(out[qc * P : (qc + 1) * P, :], o_t[:])

        nxt = emit_body(nxt, (0, 1))
        b11 = emit_body(nxt, (1, 1), boundary_wo=list(range(0, 4)))
        emit_body(b11, None, boundary_wo=list(range(4, 8)),
                  early_transpose=list(range(8, 16)), finish=False)
        # split tail: finish the q-chunk-group-0 accumulators first so their
        # evict + Wo pieces overlap the group-1 attn@V matmuls.
        rem = list(range(b11.attnv_next, KC))
        for c in rem:
            b11.attnv_chunk(c, range(0, 4))
        b11.evict(qcgs=(0,))
        emit_wo_transposes(range(8, 12), [1])
        for c in rem:
            b11.attnv_chunk(c, range(4, 8))
        emit_wo_mms(range(8, 12), scalar_copy=True)
        b11.evict(qcgs=(1,))
        emit_wo_transposes(range(12, 16), [1])
        emit_wo_mms(range(12, 16), scalar_copy=True)


_NC_CACHE = None


def _get_nc():
    global _NC_CACHE
    if _NC_CACHE is None:
        nc = bacc.Bacc("TRN2", target_bir_lowering=False, debug=False)
        with tile.TileContext(nc) as tc:
            _build_attention_kernel(tc)
        nc.compile()
        _NC_CACHE = nc
    return _NC_CACHE


def _make_in_maps(q, k, v, mask, Wq, bq, Wk, bk, Wv, bv, Wo, bo):
    import ml_dtypes
    f32 = np.float32
    f16 = np.float16
    f8 = ml_dtypes.float8_e4m3
    def to8(x):
        return np.clip(np.ascontiguousarray(x), -224, 224).astype(f8)

    qs = [np.ascontiguousarray(q[b].T).astype(f16) for b in range(B)]
    ks = [np.ascontiguousarray(k[b].T).astype(f16) for b in range(B)]
    vs = [np.ascontiguousarray(v[b].T).astype(f16) for b in range(B)]
    inv_u8 = (~np.asarray(mask).astype(bool)).view(np.uint8)
    WS = 64.0

    def rearr_w(w, dt=f8):
        # [D, C] -> [128, DC*C]: row r holds W[j*128+r, :] for j in 0..DC-1
        c = w.shape[1]
        r = np.ascontiguousarray(
            w.reshape(DC, P, c).transpose(1, 0, 2).reshape(P, DC * c)
        )
        if dt is not np.float16:
            r = np.clip(r, -224, 224)
        return r.astype(dt)

    in_maps = []
    for c in range(N_CORES):
        b, hg = c // 4, c % 4
        cs = slice(hg * HPC * DH, (hg + 1) * HPC * DH)
        wo_c = np.asarray(Wo[cs, :])  # [256, D]
        in_maps.append(
            {
                "qt": qs[b],
                "kt": ks[b],
                "vt": vs[b],
                "invm": np.ascontiguousarray(
                    inv_u8[b, hg * HPC : (hg + 1) * HPC].transpose(0, 2, 1)
                ),
                "wq": rearr_w(np.asarray(Wq[:, cs]) * SCALE, dt=f16),
                "wk": rearr_w(np.asarray(Wk[:, cs]), dt=f16),
                "wv": rearr_w(np.asarray(Wv[:, cs]), dt=f16),
                "wo": np.ascontiguousarray(
                    wo_c.reshape(PAIRS, P, D).transpose(1, 0, 2).reshape(P, -1)
                ).astype(f16),
                "bq": np.ascontiguousarray(
                    (np.asarray(bq[cs]) * SCALE).reshape(PAIRS, P).T, dtype=f32
                ),
                "bk": np.ascontiguousarray(
                    np.asarray(bk[cs]).reshape(PAIRS, P).T, dtype=f32
                ),
                "bv": np.ascontiguousarray(bv[cs]).astype(f16),
            }
        )
    return in_maps


def _assemble(results, bo):
    out = np.empty((B, S, D), dtype=np.float32)
    for b in range(B):
        acc = results[4 * b]["out"].astype(np.float32)
        for g in range(1, 4):
            acc = acc + results[4 * b + g]["out"].astype(np.float32)
        out[b] = acc + np.asarray(bo, dtype=np.float32)
    return out


def run(inputs, trace=False, tmpdir=None):
    nc = _get_nc()
    in_maps = _make_in_maps(**inputs)
    res = run_bass_kernel_spmd(
        nc, in_maps, list(range(N_CORES)), trace=trace, tmpdir=tmpdir
    )
    return _assemble(res.results, inputs["bo"]), res


def kernel(**inputs) -> np.ndarray:
    inputs = {k: np.asarray(v) for k, v in inputs.items()}
    out, _ = run(inputs)
    return out



# revision 36
# speedup vs baseline: 1.0313x; 1.0313x over previous
"""Multi-head attention (B=2, S=2048, D=1024, H=16) on 8 TRN2 NeuronCores.

Sharding: batch x head-group parallel. Core c handles batch b = c//4 and
heads 4*(c%4) .. 4*(c%4)+3.  Q/K/V projections are column-split per core,
Wo is row-split; the 4 partial [S, D] outputs per batch are summed on the
host (f16 partials, f32 accumulation + bo).

v5 design (vs. 304us baseline):
  - masks stored u8 in DRAM (16.8 MB/core) and cast u8->f16 by the SWDGE
    casting DMA; mask tiles are one TT-pair each ([128, 2, 1024], 4KB)
    with a 10-deep pool so the mask stream runs ~5 chunk-pairs ahead.
  - score matmuls for the two heads of a pair are row-tiled (K=64 rows
    0-63 / 64-127) and run concurrently in the PE array.
  - ScalarE's exp stream (~147us) is the critical engine.  The serial
    chain exp -> mask-mul -> attn@V -> next scores is broken by lagging
    attn@V two k-chunks behind the mask multiply, so the PE never waits
    on the current chunk's DVE work.
  - attn@V: pts stationary [128,128], N=65; 4 q-chunks share one PSUM
    bank ([128,4,65], Z in col 64; only the first matmul sets start=True
    since start clears the whole bank).  Eviction = reciprocal +
    stride-0-broadcast multiply into outh.
  - Wo projection pieces (transpose + 2 matmuls + copy + DMA per 128-q
    chunk) run in block-boundary windows on the freed attn@V PSUM slots;
    output partials are f16.
"""

import os
import sys

for _p in ("/opt/trn_rl_repo", "/root/.axon_site/_ro/trn_rl_repo"):
    if os.path.isdir(_p) and _p not in sys.path:
        sys.path.append(_p)

import numpy as np

import concourse.bass as bass
import concourse.tile as tile
from concourse import bacc, mybir
from concourse.bass_utils import run_bass_kernel_spmd

B, S, D, H = 2, 2048, 1024, 16
DH = D // H            # 64
HPC = 4                # heads per core
PAIRS = 2              # head pairs per core
N_CORES = 8
P = 128
NB = 512               # matmul free-dim block (one PSUM bank of fp32)
KC = S // P            # 16 k chunks
QB = 1024              # q block (2 blocks per core)
DC = D // P            # 8 contraction chunks for projections
SCALE = 1.0 / 8.0      # 1/sqrt(DH)

F32 = mybir.dt.float32
F16 = mybir.dt.float16
F8 = mybir.dt.float8e4
U8 = mybir.dt.uint8
WS = 1.0               # no weight scaling in the all-f16 configuration

Exp = mybir.ActivationFunctionType.Exp


def _build_attention_kernel(tc):
    nc = tc.nc
    qt = nc.dram_tensor("qt", [D, S], F16, kind="ExternalInput").ap()
    kt = nc.dram_tensor("kt", [D, S], F16, kind="ExternalInput").ap()
    vt = nc.dram_tensor("vt", [D, S], F16, kind="ExternalInput").ap()
    # inverted transposed mask: 1 = keep, 0 = masked; [head, k, q], u8
    invm = nc.dram_tensor("invm", [HPC, S, S], U8, kind="ExternalInput").ap()
    # host pre-rearranged weights (see _make_in_maps)
    wq = nc.dram_tensor("wq", [P, DC * HPC * DH], F16, kind="ExternalInput").ap()
    wk = nc.dram_tensor("wk", [P, DC * HPC * DH], F16, kind="ExternalInput").ap()
    wv = nc.dram_tensor("wv", [P, DC * HPC * DH], F16, kind="ExternalInput").ap()
    wo = nc.dram_tensor("wo", [P, PAIRS * D], F16, kind="ExternalInput").ap()
    bq = nc.dram_tensor("bq", [P, PAIRS], F32, kind="ExternalInput").ap()
    bk = nc.dram_tensor("bk", [P, PAIRS], F32, kind="ExternalInput").ap()
    bv = nc.dram_tensor("bv", [HPC * DH], F16, kind="ExternalInput").ap()
    out = nc.dram_tensor("out", [S, D], F16, kind="ExternalOutput").ap()

    with (
        tc.tile_pool(name="const", bufs=1) as constp,
        tc.tile_pool(name="wts", bufs=1) as wtsp,
        tc.tile_pool(name="proj", bufs=1) as projp,
        tc.tile_pool(name="xt", bufs=16) as xtp,
        tc.tile_pool(name="pts", bufs=2) as ptsp,
        tc.tile_pool(name="mask", bufs=5) as maskp,
        tc.tile_pool(name="rz", bufs=4) as rzp,
        tc.tile_pool(name="ostage", bufs=2) as ostagep,
        tc.tile_pool(name="po", bufs=4, space="PSUM") as pop,
        tc.tile_pool(name="ps2", bufs=2, space="PSUM") as psp2,
    ):
        # ---- constants ----
        ones_f = constp.tile([1, P], F32)
        nc.vector.memset(ones_f[:], 1.0)
        ones_row = constp.tile([1, P], F16)      # K=1 lhsT for v bias add
        nc.vector.memset(ones_row[:], 1.0)
        # warm-up stream: dependency-free matmuls (K=1, ones stationary) so
        # HAM lifts the PE clock to 2.4GHz before the projection matmuls
        # arrive; gated only on a memset, unlike the old identity-based
        # stream that waited ~9us for make_identity.
        warm = pop.tile([P, DH], F32, tag="po", name="warm")
        for _ in range(96):
            nc.tensor.matmul(
                warm[:], ones_row[:], ones_row[:, 0:DH], start=True,
                stop=True, skip_group_check=True,
            )
        ident = constp.tile([P, P], F16)
        from concourse.masks import make_identity
        make_identity(nc, ident[:])
        # trigger the exp ACT table load early, while DMA-bound
        dummy = constp.tile([1, P], F16)
        nc.scalar.activation(dummy[:], ones_f[:], Exp)

        # ---- weights / biases (contiguous layouts, scalar HWDGE ring) ----
        def load_w(name, w_ap, dt=F16):
            t = wtsp.tile([P, DC, HPC * DH], dt, tag=name)
            nc.scalar.dma_start(t[:], w_ap.rearrange("r (j c) -> r j c", j=DC))
            return t

        wk_sb = load_w("wk", wk)    # k first: khT projection starts first
        wq_sb = load_w("wq", wq)
        bq_sb = wtsp.tile([P, PAIRS], F32, tag="bq")
        nc.scalar.dma_start(bq_sb[:], bq)
        bk_sb = wtsp.tile([P, PAIRS], F32, tag="bk")
        nc.scalar.dma_start(bk_sb[:], bk)
        bv_sb = wtsp.tile([1, HPC * DH], F16, tag="bv")
        nc.scalar.dma_start(bv_sb[:], bv.rearrange("(o c) -> o c", o=1))
        # wv/wo loads are deferred into the first block's j-loop to keep
        # startup DMA bandwidth for the k/q/mask critical path
        wv_sb = wtsp.tile([P, DC, HPC * DH], F16, tag="wv")
        wo_sb = wtsp.tile([P, PAIRS, D], F16, tag="wo")

        def load_wv():
            nc.scalar.dma_start(
                wv_sb[:], wv.rearrange("r (j c) -> r j c", j=DC)
            )

        def load_wo():
            nc.scalar.dma_start(
                wo_sb[:], wo.rearrange("r (p n) -> r p n", p=PAIRS)
            )

        # ---- persistent activations ----
        qhT = projp.tile([P, PAIRS, S], F16, tag="qhT")
        khT = projp.tile([P, PAIRS, S], F16, tag="khT")
        vh1 = projp.tile([P, HPC, KC, DH + 1], F16, tag="vh1")
        nc.vector.memset(vh1[:, :, :, DH : DH + 1], WS)
        outh = projp.tile([P, S // P, HPC * DH], F16, tag="outh")
        outhT = projp.tile([P, PAIRS, S], F16, tag="outhT")

        # ---- q/k projection: per-S/2-half DMA load, per-512q mm piece ----
        def emit_qk_dmas(src, sh, eng=None, dt=F16, split=False, eng2=None):
            eng = eng or nc.sync
            eng2 = eng2 or eng
            c0 = sh * (S // 2)
            xts = []
            for j in range(DC):
                x_t = xtp.tile([P, S // 2], dt, name=f"x_{j}", tag="xt")
                if split:
                    # first 512 columns queued for all j first: the sq=0
                    # projection pieces (and so the first scores/exp) start
                    # after half the DMA bytes.
                    eng.dma_start(
                        x_t[:, 0:NB], src[j * P : (j + 1) * P, c0 : c0 + NB]
                    )
                else:
                    eng.dma_start(
                        x_t[:], src[j * P : (j + 1) * P, c0 : c0 + S // 2]
                    )
                xts.append(x_t)
            if split:
                for j in range(DC):
                    eng2.dma_start(
                        xts[j][:, NB : S // 2],
                        src[j * P : (j + 1) * P, c0 + NB : c0 + S // 2],
                    )
            return xts

        def emit_qk_mms(xts, w_sb, b_sb, dst, sh, sq):
            c0 = sh * (S // 2) + sq * NB
            ps = [
                pop.tile([P, NB], F32, tag="po", name=f"ps_proj{p}")
                for p in range(PAIRS)
            ]
            for j in range(DC):
                for p in range(PAIRS):
                    nc.tensor.matmul(
                        ps[p][:],
                        w_sb[:, j, p * P : (p + 1) * P],
                        xts[j][:, sq * NB : (sq + 1) * NB],
                        start=(j == 0),
                        stop=(j == DC - 1),
                    )
            for p in range(PAIRS):
                nc.vector.tensor_scalar_add(
                    dst[:, p, c0 : c0 + NB], ps[p][:], b_sb[:, p : p + 1]
                )

        # ---- v projection mms for k-chunks of one half ----
        def emit_v_mms(vts, sh, sq, kks=range(4)):
            for kk in kks:
                kidx = sh * 8 + sq * 4 + kk
                ps = psp2.tile([P, HPC * DH], F32, tag="ps2", name="ps_v")
                for j in range(DC):
                    nc.tensor.matmul(
                        ps[:],
                        vts[j][:, (sq * 4 + kk) * P : (sq * 4 + kk + 1) * P],
                        wv_sb[:, j, :],
                        start=(j == 0),
                        stop=False,
                    )
                nc.tensor.matmul(
                    ps[:], ones_row[:], bv_sb[:], start=False, stop=True
                )
                nc.vector.tensor_copy(
                    vh1[:, :, kidx, 0:DH],
                    ps[:].rearrange("r (h c) -> r h c", h=HPC),
                )

        invm_r = [invm[lh].rearrange("(j p) q -> p j q", p=P) for lh in range(HPC)]

        # ---- attention block machinery ----
        class Block:
            def __init__(self, pp, qb, eager_groups=1):
                self.pp, self.qb = pp, qb
                self.m_tiles = {}
                self.m_next = 0
                # per-chunk pts tiles: subtile tracking on a strided view of
                # one big tile collapses to whole-tile deps, which made the
                # next block's first exp wait on this block's LAST attn@V
                # reads. 17 bufs (vs 16 live) shift the slot mapping each
                # block so a new tile's slot held a long-dead chunk.
                self.pts = [
                    [
                        ptsp.tile([P, QB], F16, name=f"pts{h2}_{c}",
                                  tag=f"pts{h2}", bufs=17)
                        for c in range(KC)
                    ]
                    for h2 in range(2)
                ]
                self.po = {}       # (h2, qcg) -> psum tile
                self.attnv_next = 0
                self.ensure_masks(eager_groups)

            def ensure_masks(self, upto):
                # group-major, head-minor: slot reuse order matches the TT
                # consumption order; lazy emission keeps startup bandwidth
                # for the k/q/weight critical path.
                while self.m_next < min(upto, 4):
                    g = self.m_next
                    for h2 in range(2):
                        lh = self.pp * 2 + h2
                        m_t = maskp.tile(
                            [P, 4, QB], F16, name=f"m_{h2}_{g}", tag="m"
                        )
                        nc.gpsimd.dma_start(
                            m_t[:],
                            invm_r[lh][
                                :,
                                g * 4 : (g + 1) * 4,
                                self.qb * QB : (self.qb + 1) * QB,
                            ],
                        )
                        self.m_tiles[(h2, g)] = m_t
                    self.m_next += 1

            def scores(self, j):
                """Scores + exp for k-chunk j (both heads, row-tiled)."""
                pss = [
                    psp2.tile([P, QB], F32, tag="ps2", name=f"ps_s{h2}")
                    for h2 in range(2)
                ]
                for nh in range(2):
                    for h2 in range(2):
                        d0 = h2 * DH
                        q0 = self.qb * QB + nh * NB
                        nc.tensor.matmul(
                            pss[h2][:, nh * NB : (nh + 1) * NB],
                            khT[d0 : d0 + DH, self.pp, j * P : (j + 1) * P],
                            qhT[d0 : d0 + DH, self.pp, q0 : q0 + NB],
                            start=True,
                            stop=True,
                        )
                for h2 in range(2):
                    nc.scalar.activation(self.pts[h2][j][:], pss[h2][:], Exp)

            def mask(self, j):
                """Mask-multiply chunk pair (j-1, j), N=2048 per DVE op."""
                g, r = (j - 1) // 4, ((j - 1) % 4) // 2
                self.ensure_masks(g + 2)
                for h2 in range(2):
                    for dj in (0, 1):
                        sl = self.pts[h2][j - 1 + dj][:]
                        nc.vector.tensor_mul(
                            sl, sl,
                            self.m_tiles[(h2, g)][:, 2 * r + dj, :],
                        )

            def attnv_chunk(self, c, qc8s=range(8)):
                """attn@V matmuls for (already masked) k-chunk c."""
                for h2 in range(2):
                    lh = self.pp * 2 + h2
                    for qc8 in qc8s:
                        qcg, g = qc8 // 4, qc8 % 4
                        po = self.po.get((h2, qcg))
                        if po is None:
                            po = pop.tile(
                                [P, 4, DH + 1], F32, tag="po",
                                name=f"po{h2}{qcg}",
                            )
                            self.po[(h2, qcg)] = po
                        # start=True clears the whole PSUM bank: only the
                        # first matmul into the tile sets it; later groups'
                        # first writes use per-element has_written overwrite.
                        nc.tensor.matmul(
                            po[:, g, :],
                            self.pts[h2][c][:, qc8 * P : (qc8 + 1) * P],
                            vh1[:, lh, c, :],
                            start=(c == 0 and g == 0),
                            stop=(c == KC - 1),
                            skip_group_check=True,
                        )

            def attnv_upto(self, limit):
                while self.attnv_next < limit:
                    self.attnv_chunk(self.attnv_next)
                    self.attnv_next += 1

            def evict(self, qcgs=(0, 1)):
                """Normalize po accumulators into outh."""
                for h2 in range(2):
                    lh = self.pp * 2 + h2
                    for qcg in qcgs:
                        po = self.po[(h2, qcg)]
                        rzt = rzp.tile([P, 4], F32, name="rzt", tag="rz")
                        nc.vector.reciprocal(rzt[:], po[:, :, DH])
                        qc0 = self.qb * 8 + qcg * 4
                        nc.vector.tensor_mul(
                            outh[:, qc0 : qc0 + 4, lh * DH : (lh + 1) * DH],
                            po[:, 0:4, 0:DH],
                            rzt[:].unsqueeze(2).broadcast_to([P, 4, DH]),
                        )

        # Wo projection piece for one 128-q chunk (after outh[qc] complete).
        # psum comes from the po pool: call only in boundary/tail windows.
        def emit_wo_piece(qc):
            for p in range(PAIRS):
                tp = pop.tile([P, P], F16, tag="po", name="tp")
                nc.tensor.transpose(
                    tp[:], outh[:, qc, p * P : (p + 1) * P], ident[:]
                )
                nc.vector.tensor_copy(
                    outhT[:, p, qc * P : (qc + 1) * P], tp[:]
                )
            o_t = ostagep.tile([P, D], F16)
            for nb in range(D // NB):
                pf = pop.tile([P, NB], F32, tag="po", name="pf")
                for p in range(PAIRS):
                    nc.tensor.matmul(
                        pf[:],
                        outhT[:, p, qc * P : (qc + 1) * P],
                        wo_sb[:, p, nb * NB : (nb + 1) * NB],
                        start=(p == 0),
                        stop=(p == PAIRS - 1),
                    )
                nc.vector.tensor_copy(o_t[:, nb * NB : (nb + 1) * NB], pf[:])
            nc.sync.dma_start(out[qc * P : (qc + 1) * P, :], o_t[:])

        # ---- emission schedule ----
        # kt streams on the sync ring while qt+weights stream on the
        # scalar ring; the mask DMAs (gpsimd ring) are emitted after so
        # the k/q critical path gets first claim on the SDMA engines.
        kts = emit_qk_dmas(kt, 0, nc.sync, split=True)
        qts = emit_qk_dmas(qt, 0, nc.scalar, split=True)
        # eager_groups=1: only the first mask group-pair competes with the
        # k/q/weight streams for startup HBM bandwidth; later groups are
        # fetched lazily two groups ahead of use.
        b00 = Block(0, 0, eager_groups=1)
        emit_qk_mms(kts, wk_sb, bk_sb, khT, 0, 0)
        emit_qk_mms(qts, wq_sb, bq_sb, qhT, 0, 0)
        emit_qk_mms(kts, wk_sb, bk_sb, khT, 0, 1)
        emit_qk_mms(qts, wq_sb, bq_sb, qhT, 0, 1)

        # block (0,0): scores stream immediately; the k/q second halves
        # and the v projection are inserted piecewise into the exp-paced
        # j-loop (pieces sized ~1-3us so they never head-of-line-block the
        # exp stream for long); attn@V catches up 4 chunks per odd j from
        # j=9.
        nxt = None
        for j in range(KC):
            b00.scores(j)
            if j % 2 == 1:
                b00.mask(j)
            if j == 1:
                kts = emit_qk_dmas(kt, 1, nc.sync)
                emit_qk_mms(kts, wk_sb, bk_sb, khT, 1, 0)
            if j == 2:
                emit_qk_mms(kts, wk_sb, bk_sb, khT, 1, 1)
                load_wv()
            if j == 3:
                qts = emit_qk_dmas(qt, 1, nc.scalar)
                emit_qk_mms(qts, wq_sb, bq_sb, qhT, 1, 0)
            if j == 4:
                emit_qk_mms(qts, wq_sb, bq_sb, qhT, 1, 1)
            if j == 5:
                vts = emit_qk_dmas(vt, 0, nc.sync)
                emit_v_mms(vts, 0, 0, range(0, 2))
            if j == 6:
                emit_v_mms(vts, 0, 0, range(2, 4))
                load_wo()
            if j == 7:
                emit_v_mms(vts, 0, 1, range(0, 2))
            if j == 8:
                emit_v_mms(vts, 0, 1, range(2, 4))
                vts = emit_qk_dmas(vt, 1, nc.sync)
            if j == 9:
                emit_v_mms(vts, 1, 0, range(0, 2))
            if j == 10:
                emit_v_mms(vts, 1, 0, range(2, 4))
            if j == 11:
                emit_v_mms(vts, 1, 1, range(0, 2))
            if j == 12:
                emit_v_mms(vts, 1, 1, range(2, 4))
            if j >= 11 and j % 2 == 1:
                b00.attnv_upto(min(4 * ((j - 9) // 2), j - 1))
            if j == 13:
                nxt = Block(1, 0, eager_groups=3)
        b00.attnv_upto(KC)
        b00.evict()

        def emit_body(blk, next_spec, boundary_wo=(), finish=True,
                      early_transpose=()):
            nxt = None
            for j in range(KC):
                blk.scores(j)
                if j % 2 == 1:
                    blk.mask(j)
                    if j >= 3:
                        blk.attnv_upto(j - 2)
                if j in (1, 3) and boundary_wo:
                    # boundary window: previous q-block's Wo pieces run on
                    # the PE while this block's first exps drain.
                    half = len(boundary_wo) // 2
                    for qc in (boundary_wo[:half] if j == 1 else boundary_wo[half:]):
                        emit_wo_piece(qc)
                if j in (5, 7) and early_transpose:
                    # pair-0 columns of outh for this pair-1 block's q-range
                    # are already final: transpose them now, off the tail.
                    half = len(early_transpose) // 2
                    qcs = early_transpose[:half] if j == 5 else early_transpose[half:]
                    emit_wo_transposes(qcs, [0], use_ps2=True)
                if j == 13 and next_spec:
                    nxt = Block(*next_spec, eager_groups=3)
            if finish:
                blk.attnv_upto(KC)
                blk.evict()
            return nxt

        # transposes for outh -> outhT, one (qc, pair); pool selectable so
        # pair-0 transposes can run a block early (outh cols 0-127 are
        # written by the pair-0 blocks alone).
        def emit_wo_transposes(qcs, pairs, use_ps2=False):
            for qc in qcs:
                for p in pairs:
                    pool = psp2 if use_ps2 else pop
                    tp = pool.tile([P, P], F16,
                                   tag="ps2" if use_ps2 else "po", name="tp")
                    nc.tensor.transpose(
                        tp[:], outh[:, qc, p * P : (p + 1) * P], ident[:]
                    )
                    nc.vector.tensor_copy(
                        outhT[:, p, qc * P : (qc + 1) * P], tp[:]
                    )

        def emit_wo_mms(qcs, scalar_copy=False):
            Cp = mybir.ActivationFunctionType.Copy
            for qc in qcs:
                o_t = ostagep.tile([P, D], F16)
                for nb in range(D // NB):
                    pool = pop if nb == 0 else psp2
                    pf = pool.tile([P, NB], F32,
                                   tag="po" if pool is pop else "ps2",
                                   name="pf")
                    for p in range(PAIRS):
                        nc.tensor.matmul(
                            pf[:],
                            outhT[:, p, qc * P : (qc + 1) * P],
                            wo_sb[:, p, nb * NB : (nb + 1) * NB],
                            start=(p == 0),
                            stop=(p == PAIRS - 1),
                        )
                    dst = o_t[:, nb * NB : (nb + 1) * NB]
                    if scalar_copy and nb == 1:
                        nc.scalar.activation(dst, pf[:], Cp)
                    else:
                        nc.vector.tensor_copy(dst, pf[:])
                nc.sync.dma_start(out[qc * P : (qc + 1) * P, :], o_t[:])

        nxt = emit_body(nxt, (0, 1))
        b11 = emit_body(nxt, (1, 1), boundary_wo=list(range(0, 4)))
        emit_body(b11, None, boundary_wo=list(range(4, 8)),
                  early_transpose=list(range(8, 16)), finish=False)
        # split tail: finish the q-chunk-group-0 accumulators first so their
        # evict + Wo pieces overlap the group-1 attn@V matmuls.
        rem = list(range(b11.attnv_next, KC))
        for c in rem:
            b11.attnv_chunk(c, range(0, 4))
        b11.evict(qcgs=(0,))
        emit_wo_transposes(range(8, 12), [1])
        for c in rem:
            b11.attnv_chunk(c, range(4, 8))
        emit_wo_mms(range(8, 12), scalar_copy=True)
        b11.evict(qcgs=(1,))
        emit_wo_transposes(range(12, 16), [1])
        emit_wo_mms(range(12, 16), scalar_copy=True)


_NC_CACHE = None


def _get_nc():
    global _NC_CACHE
    if _NC_CACHE is None:
        nc = bacc.Bacc("TRN2", target_bir_lowering=False, debug=False)
        with tile.TileContext(nc) as tc:
            _build_attention_kernel(tc)
        nc.compile()
        _NC_CACHE = nc
    return _NC_CACHE


def _make_in_maps(q, k, v, mask, Wq, bq, Wk, bk, Wv, bv, Wo, bo):
    import ml_dtypes
    f32 = np.float32
    f16 = np.float16
    f8 = ml_dtypes.float8_e4m3
    def to8(x):
        return np.clip(np.ascontiguousarray(x), -224, 224).astype(f8)

    qs = [np.ascontiguousarray(q[b].T).astype(f16) for b in range(B)]
    ks = [np.ascontiguousarray(k[b].T).astype(f16) for b in range(B)]
    vs = [np.ascontiguousarray(v[b].T).astype(f16) for b in range(B)]
    inv_u8 = (~np.asarray(mask).astype(bool)).view(np.uint8)
    WS = 64.0

    def rearr_w(w, dt=f8):
        # [D, C] -> [128, DC*C]: row r holds W[j*128+r, :] for j in 0..DC-1
        c = w.shape[1]
        r = np.ascontiguousarray(
            w.reshape(DC, P, c).transpose(1, 0, 2).reshape(P, DC * c)
        )
        if dt is not np.float16:
            r = np.clip(r, -224, 224)
        return r.astype(dt)

    in_maps = []
    for c in range(N_CORES):
        b, hg = c // 4, c % 4
        cs = slice(hg * HPC * DH, (hg + 1) * HPC * DH)
        wo_c = np.asarray(Wo[cs, :])  # [256, D]
        in_maps.append(
            {
                "qt": qs[b],
                "kt": ks[b],
                "vt": vs[b],
                "invm": np.ascontiguousarray(
                    inv_u8[b, hg * HPC : (hg + 1) * HPC].transpose(0, 2, 1)
                ),
                "wq": rearr_w(np.asarray(Wq[:, cs]) * SCALE, dt=f16),
                "wk": rearr_w(np.asarray(Wk[:, cs]), dt=f16),
                "wv": rearr_w(np.asarray(Wv[:, cs]), dt=f16),
                "wo": np.ascontiguousarray(
                    wo_c.reshape(PAIRS, P, D).transpose(1, 0, 2).reshape(P, -1)
                ).astype(f16),
                "bq": np.ascontiguousarray(
                    (np.asarray(bq[cs]) * SCALE).reshape(PAIRS, P).T, dtype=f32
                ),
                "bk": np.ascontiguousarray(
                    np.asarray(bk[cs]).reshape(PAIRS, P).T, dtype=f32
                ),
                "bv": np.ascontiguousarray(bv[cs]).astype(f16),
            }
        )
    return in_maps


def _assemble(results, bo):
    out = np.empty((B, S, D), dtype=np.float32)
    for b in range(B):
        acc = results[4 * b]["out"].astype(np.float32)
        for g in range(1, 4):
            acc = acc + results[4 * b + g]["out"].astype(np.float32)
        out[b] = acc + np.asarray(bo, dtype=np.float32)
    return out


def run(inputs, trace=False, tmpdir=None):
    nc = _get_nc()
    in_maps = _make_in_maps(**inputs)
    res = run_bass_kernel_spmd(
        nc, in_maps, list(range(N_CORES)), trace=trace, tmpdir=tmpdir
    )
    return _assemble(res.results, inputs["bo"]), res


def kernel(**inputs) -> np.ndarray:
    inputs = {k: np.asarray(v) for k, v in inputs.items()}
    out, _ = run(inputs)
    return out

